# revision 23
# baseline (speedup 1.0000x reference)
"""AttentionGRU Trainium2 kernel: 8-core data-parallel over batch.

Reference computation (per example):
  xg = x @ w_ih.T + b_ih                      # hoisted input GEMM, [S, 3H]
  per step t: hg = h @ w_hh.T + b_hh
              r = sigmoid(xg_r + hg_r); z = sigmoid(xg_z + hg_z)
              n = tanh(xg_n + r * hg_n); h = (1-z)*n + z*h
  logits = out @ w_attn.T (+b_attn, softmax-invariant -> dropped)
  attn = softmax over seq; context = sum(attn * out); y = context @ w_fc.T + b_fc

Device layout (per core, B=32 examples):
  - h kept as [H=64 partitions, b free]; gates as [gate, b]. The recurrence
    is latency-bound (~2.3us/step chain of 7 instructions); one fused
    32-example chain beat dual 16-example chains because Tile's in-order
    sequencers head-of-line block on multi-producer waits and DVE pays
    ~170ns fixed cost per op.
  - Phase 1: xT (host-pretransposed, t-major tokens) [128(i), B*S] ->
    xg[g, t*B+b] via two constant stationaries; biases folded via
    per-partition bias ops; xg stored bf16 in four per-128-step tiles. The
    first 8 GEMM chunks are emitted up front, the remaining 24 interleaved
    into the recurrence emission (2 per 16 steps) and the PSUM pool is
    shared across phases 1+2, so the input GEMM tail overlaps the
    recurrence instead of serializing at the pool/tile boundaries.
  - Phase 2 per step: PE computes w_hh gates (b_hh_n via ones-row-augmented
    h) + identity-accumulate of xg into PSUM; ACT sigmoid straight from
    PSUM; DVE p = r*hn, q = p + xn; ACT tanh; DVE m1 = (1-z)*n,
    h' = m1 + z*h, with u = 1-z and m2 = z*h computed inside the tanh
    window (m2 reads h in DVE program order before h's update, so the WAR
    needs no semaphore). z is moved to partitions 0:63 by an identity-slice
    matmul (walrus requires equal SBUF input base partitions). Logits
    l_t = w_attn . h_t via a 1-column matmul into a PSUM strip flushed to
    DRAM every 32 steps; h_t history rows (gpsimd snapshot) DMA'd to
    [t, h*32+b] tiles.
  - Phase 3: softmax on [b, t], PE-transpose of attn, per-example
    accumulated matmuls for context, final FC with bias via an augmented
    ones-row.
"""

import sys

sys.path.insert(0, "/opt/trn_rl_repo")

import numpy as np

import concourse.bacc as bacc
from concourse.bass import _add_dep_helper
import concourse.tile as tile
from concourse import mybir
from concourse import bass_utils

F32 = mybir.dt.float32
BF16 = mybir.dt.bfloat16
AF = mybir.ActivationFunctionType
ALU = mybir.AluOpType

H = 64
I = 128
G = 3 * H  # 192
C = 2
N_CORES = 8
NCH = 1  # independent batch chains per core


def build_program(S: int, B: int = 32, num_devices: int = N_CORES):
    TOK = B * S
    BC = B // NCH  # examples per chain
    nc = bacc.Bacc(
        "TRN2", target_bir_lowering=False, debug=False, num_devices=num_devices
    )

    xT_d = nc.dram_tensor("xT", [I, TOK], F32, kind="ExternalInput")
    w_ihT_d = nc.dram_tensor("w_ihT", [I, G], F32, kind="ExternalInput")
    w_hhT_d = nc.dram_tensor("w_hhT_aug", [H + 1, G], F32, kind="ExternalInput")
    bias_rz_d = nc.dram_tensor("bias_rz", [2 * H, 1], F32, kind="ExternalInput")
    bias_n_d = nc.dram_tensor("bias_n", [H, 1], F32, kind="ExternalInput")
    ident_d = nc.dram_tensor("ident", [128, 128], F32, kind="ExternalInput")
    wattn_d = nc.dram_tensor("w_attn_col", [H, 1], F32, kind="ExternalInput")
    wfc_d = nc.dram_tensor("w_fcT_aug", [H + 1, C], F32, kind="ExternalInput")
    y_d = nc.dram_tensor("y", [B, C], F32, kind="ExternalOutput")
    l_ds = [
        nc.dram_tensor(f"l_scratch{ch}", [1, BC * S], F32, kind="Internal")
        for ch in range(NCH)
    ]

    n_tchunk = (S + 127) // 128  # 128-step history chunks
    assert S % 32 == 0

    with tile.TileContext(nc) as tc:
        with (
            tc.tile_pool(name="const", bufs=1) as const,
            tc.tile_pool(name="share", bufs=1) as share,
            tc.tile_pool(name="xg", bufs=1) as xgp,
            tc.tile_pool(name="sm", bufs=1) as smp,
            tc.tile_pool(name="step", bufs=4) as sp,
            tc.tile_pool(name="snap", bufs=4) as snapp,
            tc.tile_pool(name="p3", bufs=1) as p3,
        ):
            # ---- constants ----
            w_ihT = const.tile([I, G], F32)
            nc.sync.dma_start(out=w_ihT, in_=w_ihT_d.ap())
            w_hhT = const.tile([H + 1, G], F32)
            nc.sync.dma_start(out=w_hhT, in_=w_hhT_d.ap())
            bias_rz = const.tile([2 * H, 1], F32)
            nc.sync.dma_start(out=bias_rz, in_=bias_rz_d.ap())
            bias_n = const.tile([H, 1], F32)
            nc.sync.dma_start(out=bias_n, in_=bias_n_d.ap())
            ident = const.tile([128, 128], F32)
            nc.sync.dma_start(out=ident, in_=ident_d.ap())
            wattn = const.tile([H, 1], F32)
            nc.sync.dma_start(out=wattn, in_=wattn_d.ap())
            wfc = const.tile([H + 1, C], F32)
            nc.sync.dma_start(out=wfc, in_=wfc_d.ap())
            ident_bf = const.tile([128, 128], BF16)
            nc.vector.tensor_copy(ident_bf, ident)

            # ---- xT load (shares slot with history later) ----
            xT = share.tile([I, TOK], F32, tag="big")
            n_ld = max(1, TOK // 1024)
            for c in range(n_ld):
                sl = slice(c * (TOK // n_ld), (c + 1) * (TOK // n_ld))
                nc.sync.dma_start(out=xT[:, sl], in_=xT_d.ap()[:, sl])

            # xg split into per-128-step tiles: phase-2 steps in t-chunk c
            # depend only on tile c, so the recurrence starts as soon as the
            # first GEMM chunk lands instead of after the whole input GEMM
            n_tch = (S + 127) // 128
            TCH = TOK // n_tch
            xg_rz_t = [
                xgp.tile([2 * H, TCH], BF16, name=f"xg_rz{c}") for c in range(n_tch)
            ]
            xg_n_t = [
                xgp.tile([H, TCH], BF16, name=f"xg_n{c}") for c in range(n_tch)
            ]

            # ---- phase 1: input GEMM ----
            n_ck = TOK // 512
            psp12_cm = tc.tile_pool(name="ps12", bufs=1, space="PSUM")
            psp1 = psp12_cm.__enter__()
            ck_per_tile = n_ck // n_tch

            def emit_gemm_chunk(c):
                sl = slice(c * 512, (c + 1) * 512)
                ps_rz1 = psp1.tile(
                    [2 * H, 512], F32, tag="rz", bufs=2, name=f"ps_rz1_{c}"
                )
                nc.tensor.matmul(
                    ps_rz1, lhsT=w_ihT[:, 0 : 2 * H], rhs=xT[:, sl],
                    start=True, stop=True,
                )
                ps_n1 = psp1.tile([H, 512], F32, tag="n", bufs=1, name=f"ps_n1_{c}")
                nc.tensor.matmul(
                    ps_n1, lhsT=w_ihT[:, 2 * H : G], rhs=xT[:, sl],
                    start=True, stop=True,
                )
                dst = slice((c % ck_per_tile) * 512, (c % ck_per_tile + 1) * 512)
                nc.scalar.activation(
                    xg_rz_t[c // ck_per_tile][:, dst], ps_rz1, AF.Identity,
                    bias=bias_rz, scale=1.0,
                )
                nc.vector.tensor_scalar_add(
                    xg_n_t[c // ck_per_tile][:, dst], ps_n1, bias_n
                )

            # head start: first t-chunk of xg up front; the rest of the input
            # GEMM is emitted interleaved into the recurrence (2 chunks per 16
            # steps) so it rides the recurrence's idle engine slots
            next_chunk = ck_per_tile
            for c in range(ck_per_tile):
                emit_gemm_chunk(c)

            # xg views per tile: [gate, t_local, chain, b] (t-major tokens)
            xg_rz_v = [
                x.rearrange("g (s c b) -> g s c b", c=NCH, s=S // n_tch)
                for x in xg_rz_t
            ]
            xg_n_v = [
                x.rearrange("g (s c b) -> g s c b", c=NCH, s=S // n_tch)
                for x in xg_n_t
            ]

            # ---- phase 2: recurrence (NCH interleaved chains) ----
            # history rows: [t_mod, chunk, chain*1024 + h*BC + b]
            hist = xgp.tile([128, n_tchunk, NCH, H * BC], F32)
            h_aug = [smp.tile([H + 1, BC], F32, tag=f"h{ch}", name=f"h_aug{ch}") for ch in range(NCH)]
            for ch in range(NCH):
                nc.vector.memset(h_aug[ch][0:H], 0.0)
                nc.vector.memset(h_aug[ch][H : H + 1], 1.0)

            psp2 = psp1  # same pool: no pool-boundary barrier between phases
            if True:
                ps_l = [None] * NCH
                for t in range(S):
                    ps_rz, ps_n, ps_z, rz = [], [], [], []
                    # PE front: same stationary back-to-back across chains
                    for ch in range(NCH):
                        ps_rz.append(psp2.tile([2 * H, BC], F32, tag=f"psrz{ch}", name=f"ps_rz{ch}"))
                        nc.tensor.matmul(
                            ps_rz[ch], lhsT=w_hhT[:, 0 : 2 * H], rhs=h_aug[ch],
                            start=True, stop=False,
                        )
                    for ch in range(NCH):
                        nc.tensor.matmul(
                            ps_rz[ch], lhsT=ident_bf, rhs=xg_rz_v[t // (S // n_tch)][:, t % (S // n_tch), ch, :],
                            start=False, stop=True,
                        )
                    for ch in range(NCH):
                        ps_n.append(psp2.tile([H, BC], F32, tag=f"psn{ch}", name=f"ps_n{ch}"))
                        nc.tensor.matmul(
                            ps_n[ch], lhsT=w_hhT[:, 2 * H : G], rhs=h_aug[ch],
                            start=True, stop=True,
                        )
                    for ch in range(NCH):
                        r_t = sp.tile([2 * H, BC], F32, tag=f"rz{ch}")
                        nc.scalar.activation(r_t, ps_rz[ch], AF.Sigmoid)
                        rz.append(r_t)
                    for ch in range(NCH):
                        ps_z.append(psp2.tile([H, BC], F32, tag=f"psz{ch}", name=f"ps_z{ch}"))
                        nc.tensor.matmul(
                            ps_z[ch], lhsT=ident[:, H : 2 * H], rhs=rz[ch],
                            start=True, stop=True,
                        )
                    p, q, nt, i_q = [], [], [], []
                    for ch in range(NCH):
                        p.append(sp.tile([H, BC], F32, tag=f"p{ch}", name=f"p{ch}"))
                        nc.vector.tensor_mul(p[ch], rz[ch][0:H], ps_n[ch])
                        q.append(sp.tile([H, BC], F32, tag=f"q{ch}", name=f"q{ch}"))
                        i_q.append(nc.vector.tensor_add(q[ch], p[ch], xg_n_v[t // (S // n_tch)][:, t % (S // n_tch), ch, :]))
                    for ch in range(NCH):
                        nt.append(sp.tile([H, BC], F32, tag=f"nt{ch}", name=f"nt{ch}"))
                        nc.scalar.activation(nt[ch], q[ch], AF.Tanh)
                    for ch in range(NCH):
                        # u = 1-z and m2 = z*h_prev run in the tanh window
                        # (m2 reads h_aug in DVE program order before hadd's
                        # write, so the WAR needs no semaphore); only
                        # m1 = u*n and h' = m1+m2 sit after tanh on the chain
                        u = sp.tile([H, BC], F32, tag=f"u{ch}", name=f"u{ch}")
                        i_u = nc.vector.tensor_scalar(
                            u, ps_z[ch], -1.0, 1.0, op0=ALU.mult, op1=ALU.add
                        )
                        # keep the off-chain u/m2 behind q in the DVE stream so
                        # they fill the tanh window instead of delaying it
                        _add_dep_helper(i_u.ins, i_q[ch].ins, sync=False,
                                        reason="order u after q")
                        m2 = sp.tile([H, BC], F32, tag=f"m2{ch}", name=f"m2{ch}")
                        i_m2 = nc.vector.tensor_mul(m2, ps_z[ch], h_aug[ch][0:H])
                        _add_dep_helper(i_m2.ins, i_u.ins, sync=False,
                                        reason="order m2 after u")
                        m1 = sp.tile([H, BC], F32, tag=f"m1{ch}", name=f"m1{ch}")
                        nc.vector.tensor_mul(m1, u, nt[ch])
                        nc.vector.tensor_add(h_aug[ch][0:H], m1, m2)
                    for ch in range(NCH):
                        # snapshot h_t (feeds history DMA, logits, next d)
                        hs = snapp.tile([H, BC], F32, tag=f"hs{ch}")
                        nc.gpsimd.tensor_copy(hs, h_aug[ch][0:H])
                        nc.sync.dma_start(
                            out=hist[t % 128 : t % 128 + 1, t // 128, ch, :],
                            in_=hs,
                        )
                        if t % 32 == 0:
                            ps_l[ch] = psp2.tile([1, BC * 32], F32, tag=f"psl{ch}", name=f"ps_l{ch}")
                        nc.tensor.matmul(
                            ps_l[ch][:, (t % 32) * BC : (t % 32 + 1) * BC],
                            lhsT=wattn, rhs=h_aug[ch][0:H], start=True, stop=True,
                        )
                        if t % 32 == 31:
                            blk = t // 32
                            l_sb = sp.tile([1, BC * 32], F32, tag=f"lsb{ch}")
                            nc.scalar.activation(l_sb, ps_l[ch], AF.Identity)
                            nc.sync.dma_start(
                                out=l_ds[ch].ap()[
                                    :, blk * BC * 32 : (blk + 1) * BC * 32
                                ],
                                in_=l_sb,
                            )
                    if t % 16 == 15:
                        for _ in range(2):
                            if next_chunk < n_ck:
                                emit_gemm_chunk(next_chunk)
                                next_chunk += 1

            psp12_cm.__exit__(None, None, None)

            # ---- phase 3: softmax + context + fc ----
            with tc.tile_pool(name="ps3", bufs=2, space="PSUM") as psp3:
                l_bt = p3.tile([B, S], F32)
                for ch in range(NCH):
                    nc.sync.dma_start(
                        out=l_bt[ch * BC : (ch + 1) * BC],
                        in_=l_ds[ch].ap().rearrange("o (s b) -> (o b) s", b=BC),
                    )
                mx = p3.tile([B, 1], F32)
                nc.vector.reduce_max(mx, l_bt, axis=mybir.AxisListType.X, negate=True)
                e_bt = p3.tile([B, S], F32)
                ssum = p3.tile([B, 1], F32)
                nc.scalar.activation(
                    e_bt, l_bt, AF.Exp, bias=mx, scale=1.0, accum_out=ssum
                )
                rinv = p3.tile([B, 1], F32)
                nc.vector.reciprocal(rinv, ssum)
                attn = p3.tile([B, S], F32)
                nc.vector.tensor_scalar_mul(attn, e_bt, rinv)

                attn_tb = []
                for c in range(n_tchunk):
                    ps_tr = psp3.tile([128, B], F32, tag="pstr")
                    nc.tensor.transpose(
                        ps_tr, attn[:, c * 128 : (c + 1) * 128], ident[0:B, 0:B]
                    )
                    a_tb = p3.tile([128, B], F32, tag=f"atb{c}")
                    nc.vector.tensor_copy(a_tb, ps_tr)
                    attn_tb.append(a_tb)

                hist_v = hist.rearrange("p c ch (h b) -> p c ch h b", b=BC)
                ctx_ps = psp3.tile([H, B], F32, tag="ctx")
                for b in range(B):
                    ch, b16 = b // BC, b % BC
                    for c in range(n_tchunk):
                        nc.tensor.matmul(
                            ctx_ps[:, b : b + 1],
                            lhsT=hist_v[:, c, ch, :, b16],
                            rhs=attn_tb[c][:, b : b + 1],
                            start=(c == 0),
                            stop=(c == n_tchunk - 1),
                        )
                ctx_aug = p3.tile([H + 1, B], F32)
                nc.vector.memset(ctx_aug[H : H + 1], 1.0)
                nc.vector.tensor_copy(ctx_aug[0:H], ctx_ps)
                y_ps = psp3.tile([C, B], F32, tag="y")
                nc.tensor.matmul(y_ps, lhsT=wfc, rhs=ctx_aug, start=True, stop=True)
                y_sb = p3.tile([C, B], F32)
                nc.vector.tensor_copy(y_sb, y_ps)
                nc.sync.dma_start(out=y_d.ap().rearrange("b c -> c b"), in_=y_sb)

    nc.compile()
    return nc


def prep_core_inputs(x_shard, w_ih, w_hh, b_ih, b_hh, w_attn, w_fc, b_fc):
    """Build the per-core in_map from a [B, S, I] f32 shard + full params."""
    B, S, I_ = x_shard.shape
    # t-major token order [i, t*B + b]: phase-2's step-t slice is contiguous
    # and phase 1 produces early timesteps first (lets the recurrence start
    # while the input GEMM tail is still running)
    xT = np.ascontiguousarray(
        x_shard.transpose(2, 1, 0).reshape(I_, B * S), dtype=np.float32
    )
    w_hhT_aug = np.zeros((H + 1, G), dtype=np.float32)
    w_hhT_aug[0:H, :] = w_hh.T
    w_hhT_aug[H, 2 * H : G] = b_hh[2 * H : G]  # b_hh_n via ones-row
    bias_rz = (b_ih[0 : 2 * H] + b_hh[0 : 2 * H]).reshape(2 * H, 1)
    bias_n = b_ih[2 * H : G].reshape(H, 1)
    w_fcT_aug = np.zeros((H + 1, C), dtype=np.float32)
    w_fcT_aug[0:H, :] = w_fc.T
    w_fcT_aug[H, :] = b_fc
    return {
        "xT": xT,
        "w_ihT": np.ascontiguousarray(w_ih.T, dtype=np.float32),
        "w_hhT_aug": w_hhT_aug,
        "bias_rz": np.ascontiguousarray(bias_rz, dtype=np.float32),
        "bias_n": np.ascontiguousarray(bias_n, dtype=np.float32),
        "ident": np.eye(128, dtype=np.float32),
        "w_attn_col": np.ascontiguousarray(w_attn.T, dtype=np.float32),
        "w_fcT_aug": w_fcT_aug,
    }


_NC_CACHE = {}


def kernel(x, w_ih, w_hh, b_ih, b_hh, w_attn, b_attn, w_fc, b_fc):
    x = np.asarray(x, dtype=np.float32)
    w_ih = np.asarray(w_ih, dtype=np.float32)
    w_hh = np.asarray(w_hh, dtype=np.float32)
    b_ih = np.asarray(b_ih, dtype=np.float32)
    b_hh = np.asarray(b_hh, dtype=np.float32)
    w_attn = np.asarray(w_attn, dtype=np.float32)
    w_fc = np.asarray(w_fc, dtype=np.float32)
    b_fc = np.asarray(b_fc, dtype=np.float32)

    Bfull, S, _ = x.shape
    B = Bfull // N_CORES
    key = (S, B)
    if key not in _NC_CACHE:
        _NC_CACHE[key] = build_program(S, B, num_devices=N_CORES)
    nc = _NC_CACHE[key]

    in_maps = []
    for c in range(N_CORES):
        shard = x[c * B : (c + 1) * B]
        in_maps.append(
            prep_core_inputs(shard, w_ih, w_hh, b_ih, b_hh, w_attn, w_fc, b_fc)
        )
    res = bass_utils.run_bass_kernel_spmd(nc, in_maps, core_ids=list(range(N_CORES)))
    out = np.concatenate([res.results[c]["y"] for c in range(N_CORES)], axis=0)
    return out.astype(np.float32)


# revision 24
# speedup vs baseline: 1.0211x; 1.0211x over previous
"""AttentionGRU Trainium2 kernel: 8-core data-parallel over batch.

Reference computation (per example):
  xg = x @ w_ih.T + b_ih                      # hoisted input GEMM, [S, 3H]
  per step t: hg = h @ w_hh.T + b_hh
              r = sigmoid(xg_r + hg_r); z = sigmoid(xg_z + hg_z)
              n = tanh(xg_n + r * hg_n); h = (1-z)*n + z*h
  logits = out @ w_attn.T (+b_attn, softmax-invariant -> dropped)
  attn = softmax over seq; context = sum(attn * out); y = context @ w_fc.T + b_fc

Device layout (per core, B=32 examples):
  - h kept as [H=64 partitions, b free]; gates as [gate, b]. The recurrence
    is latency-bound (~2.3us/step chain of 7 instructions); one fused
    32-example chain beat dual 16-example chains because Tile's in-order
    sequencers head-of-line block on multi-producer waits and DVE pays
    ~170ns fixed cost per op.
  - Phase 1: xT (host-pretransposed, t-major tokens) [128(i), B*S] ->
    xg[g, t*B+b] via two constant stationaries; biases folded via
    per-partition bias ops; xg stored bf16 in four per-128-step tiles. The
    first 8 GEMM chunks are emitted up front, the remaining 24 interleaved
    into the recurrence emission (2 per 16 steps) and the PSUM pool is
    shared across phases 1+2, so the input GEMM tail overlaps the
    recurrence instead of serializing at the pool/tile boundaries.
  - Phase 2 per step: PE computes w_hh gates (b_hh_n via ones-row-augmented
    h) + identity-accumulate of xg into PSUM; ACT sigmoid straight from
    PSUM; DVE p = r*hn, q = p + xn; ACT tanh; DVE m1 = (1-z)*n,
    h' = m1 + z*h, with u = 1-z and m2 = z*h computed inside the tanh
    window (m2 reads h in DVE program order before h's update, so the WAR
    needs no semaphore). z is moved to partitions 0:63 by an identity-slice
    matmul (walrus requires equal SBUF input base partitions). Logits
    l_t = w_attn . h_t via a 1-column matmul into a PSUM strip flushed to
    DRAM every 32 steps; h_t history rows (gpsimd snapshot) DMA'd to
    [t, h*32+b] tiles.
  - Phase 3: softmax on [b, t], PE-transpose of attn, per-example
    accumulated matmuls for context, final FC with bias via an augmented
    ones-row.
"""

import sys

sys.path.insert(0, "/opt/trn_rl_repo")

import numpy as np

import concourse.bacc as bacc
from concourse.bass import _add_dep_helper
import concourse.tile as tile
from concourse import mybir
from concourse import bass_utils

F32 = mybir.dt.float32
BF16 = mybir.dt.bfloat16
AF = mybir.ActivationFunctionType
ALU = mybir.AluOpType

H = 64
I = 128
G = 3 * H  # 192
C = 2
N_CORES = 8
NCH = 1  # independent batch chains per core


def build_program(S: int, B: int = 32, num_devices: int = N_CORES):
    TOK = B * S
    BC = B // NCH  # examples per chain
    nc = bacc.Bacc(
        "TRN2", target_bir_lowering=False, debug=False, num_devices=num_devices
    )

    xT_d = nc.dram_tensor("xT", [I, TOK], F32, kind="ExternalInput")
    w_ihT_d = nc.dram_tensor("w_ihT", [I, G], F32, kind="ExternalInput")
    w_hhT_d = nc.dram_tensor("w_hhT_aug", [H + 1, G], F32, kind="ExternalInput")
    bias_rz_d = nc.dram_tensor("bias_rz", [2 * H, 1], F32, kind="ExternalInput")
    bias_n_d = nc.dram_tensor("bias_n", [H, 1], F32, kind="ExternalInput")
    ident_d = nc.dram_tensor("ident", [128, 128], F32, kind="ExternalInput")
    wattn_d = nc.dram_tensor("w_attn_col", [H, 1], F32, kind="ExternalInput")
    wfc_d = nc.dram_tensor("w_fcT_aug", [H + 1, C], F32, kind="ExternalInput")
    y_d = nc.dram_tensor("y", [B, C], F32, kind="ExternalOutput")
    l_ds = [
        nc.dram_tensor(f"l_scratch{ch}", [1, BC * S], F32, kind="Internal")
        for ch in range(NCH)
    ]

    n_tchunk = (S + 127) // 128  # 128-step history chunks
    assert S % 32 == 0

    with tile.TileContext(nc) as tc:
        with (
            tc.tile_pool(name="const", bufs=1) as const,
            tc.tile_pool(name="share", bufs=1) as share,
            tc.tile_pool(name="xg", bufs=1) as xgp,
            tc.tile_pool(name="sm", bufs=1) as smp,
            tc.tile_pool(name="step", bufs=4) as sp,
            tc.tile_pool(name="snap", bufs=4) as snapp,
            tc.tile_pool(name="p3", bufs=1) as p3,
        ):
            # ---- constants ----
            w_ihT = const.tile([I, G], F32)
            nc.sync.dma_start(out=w_ihT, in_=w_ihT_d.ap())
            w_hhT = const.tile([H + 1, G], F32)
            nc.sync.dma_start(out=w_hhT, in_=w_hhT_d.ap())
            bias_rz = const.tile([2 * H, 1], F32)
            nc.sync.dma_start(out=bias_rz, in_=bias_rz_d.ap())
            bias_n = const.tile([H, 1], F32)
            nc.sync.dma_start(out=bias_n, in_=bias_n_d.ap())
            ident = const.tile([128, 128], F32)
            nc.sync.dma_start(out=ident, in_=ident_d.ap())
            wattn = const.tile([H, 1], F32)
            nc.sync.dma_start(out=wattn, in_=wattn_d.ap())
            wfc = const.tile([H + 1, C], F32)
            nc.sync.dma_start(out=wfc, in_=wfc_d.ap())
            ident_bf = const.tile([128, 128], BF16)
            nc.vector.tensor_copy(ident_bf, ident)

            # ---- xT load (shares slot with history later) ----
            xT = share.tile([I, TOK], F32, tag="big")
            n_ld = max(1, TOK // 1024)
            for c in range(n_ld):
                sl = slice(c * (TOK // n_ld), (c + 1) * (TOK // n_ld))
                nc.sync.dma_start(out=xT[:, sl], in_=xT_d.ap()[:, sl])

            # xg split into per-128-step tiles: phase-2 steps in t-chunk c
            # depend only on tile c, so the recurrence starts as soon as the
            # first GEMM chunk lands instead of after the whole input GEMM
            n_tch = (S + 127) // 128
            TCH = TOK // n_tch
            xg_rz_t = [
                xgp.tile([2 * H, TCH], BF16, name=f"xg_rz{c}") for c in range(n_tch)
            ]
            xg_n_t = [
                xgp.tile([H, TCH], BF16, name=f"xg_n{c}") for c in range(n_tch)
            ]

            # ---- phase 1: input GEMM ----
            n_ck = TOK // 512
            psp12_cm = tc.tile_pool(name="ps12", bufs=1, space="PSUM")
            psp1 = psp12_cm.__enter__()
            ck_per_tile = n_ck // n_tch

            def emit_gemm_chunk(c):
                sl = slice(c * 512, (c + 1) * 512)
                ps_rz1 = psp1.tile(
                    [2 * H, 512], F32, tag="rz", bufs=2, name=f"ps_rz1_{c}"
                )
                nc.tensor.matmul(
                    ps_rz1, lhsT=w_ihT[:, 0 : 2 * H], rhs=xT[:, sl],
                    start=True, stop=True,
                )
                ps_n1 = psp1.tile([H, 512], F32, tag="n", bufs=1, name=f"ps_n1_{c}")
                nc.tensor.matmul(
                    ps_n1, lhsT=w_ihT[:, 2 * H : G], rhs=xT[:, sl],
                    start=True, stop=True,
                )
                dst = slice((c % ck_per_tile) * 512, (c % ck_per_tile + 1) * 512)
                nc.scalar.activation(
                    xg_rz_t[c // ck_per_tile][:, dst], ps_rz1, AF.Identity,
                    bias=bias_rz, scale=1.0,
                )
                nc.vector.tensor_scalar_add(
                    xg_n_t[c // ck_per_tile][:, dst], ps_n1, bias_n
                )

            # head start: first t-chunk of xg up front; the rest of the input
            # GEMM is emitted interleaved into the recurrence (2 chunks per 16
            # steps) so it rides the recurrence's idle engine slots
            next_chunk = ck_per_tile
            for c in range(ck_per_tile):
                emit_gemm_chunk(c)

            # xg views per tile: [gate, t_local, chain, b] (t-major tokens)
            xg_rz_v = [
                x.rearrange("g (s c b) -> g s c b", c=NCH, s=S // n_tch)
                for x in xg_rz_t
            ]
            xg_n_v = [
                x.rearrange("g (s c b) -> g s c b", c=NCH, s=S // n_tch)
                for x in xg_n_t
            ]

            # ---- phase 2: recurrence (NCH interleaved chains) ----
            # history rows: [t_mod, chunk, chain*1024 + h*BC + b]
            hist = xgp.tile([128, n_tchunk, NCH, H * BC], F32)
            h_aug = [smp.tile([H + 1, BC], F32, tag=f"h{ch}", name=f"h_aug{ch}") for ch in range(NCH)]
            for ch in range(NCH):
                nc.vector.memset(h_aug[ch][0:H], 0.0)
                nc.vector.memset(h_aug[ch][H : H + 1], 1.0)

            psp2 = psp1  # same pool: no pool-boundary barrier between phases
            if True:
                ps_l = [None] * NCH
                for t in range(S):
                    ps_rz, ps_n, ps_z, rz = [], [], [], []
                    # PE front: same stationary back-to-back across chains
                    for ch in range(NCH):
                        # xg-accumulate first: it has no dependency on h, so
                        # the PE runs it in the previous step's idle window and
                        # only the 53ns W.h matmul sits between hadd and sigmoid
                        ps_rz.append(psp2.tile([2 * H, BC], F32, tag=f"psrz{ch}", name=f"ps_rz{ch}"))
                        nc.tensor.matmul(
                            ps_rz[ch], lhsT=ident_bf, rhs=xg_rz_v[t // (S // n_tch)][:, t % (S // n_tch), ch, :],
                            start=True, stop=False,
                        )
                    for ch in range(NCH):
                        nc.tensor.matmul(
                            ps_rz[ch], lhsT=w_hhT[:, 0 : 2 * H], rhs=h_aug[ch],
                            start=False, stop=True,
                        )
                    for ch in range(NCH):
                        ps_n.append(psp2.tile([H, BC], F32, tag=f"psn{ch}", name=f"ps_n{ch}"))
                        nc.tensor.matmul(
                            ps_n[ch], lhsT=w_hhT[:, 2 * H : G], rhs=h_aug[ch],
                            start=True, stop=True,
                        )
                    for ch in range(NCH):
                        r_t = sp.tile([2 * H, BC], F32, tag=f"rz{ch}")
                        nc.scalar.activation(r_t, ps_rz[ch], AF.Sigmoid)
                        rz.append(r_t)
                    for ch in range(NCH):
                        ps_z.append(psp2.tile([H, BC], F32, tag=f"psz{ch}", name=f"ps_z{ch}"))
                        nc.tensor.matmul(
                            ps_z[ch], lhsT=ident[:, H : 2 * H], rhs=rz[ch],
                            start=True, stop=True,
                        )
                    p, q, nt, i_q = [], [], [], []
                    for ch in range(NCH):
                        p.append(sp.tile([H, BC], F32, tag=f"p{ch}", name=f"p{ch}"))
                        nc.vector.tensor_mul(p[ch], rz[ch][0:H], ps_n[ch])
                        q.append(sp.tile([H, BC], F32, tag=f"q{ch}", name=f"q{ch}"))
                        i_q.append(nc.vector.tensor_add(q[ch], p[ch], xg_n_v[t // (S // n_tch)][:, t % (S // n_tch), ch, :]))
                    for ch in range(NCH):
                        nt.append(sp.tile([H, BC], F32, tag=f"nt{ch}", name=f"nt{ch}"))
                        nc.scalar.activation(nt[ch], q[ch], AF.Tanh)
                    for ch in range(NCH):
                        # u = 1-z and m2 = z*h_prev run in the tanh window
                        # (m2 reads h_aug in DVE program order before hadd's
                        # write, so the WAR needs no semaphore); only
                        # m1 = u*n and h' = m1+m2 sit after tanh on the chain
                        u = sp.tile([H, BC], F32, tag=f"u{ch}", name=f"u{ch}")
                        i_u = nc.vector.tensor_scalar(
                            u, ps_z[ch], -1.0, 1.0, op0=ALU.mult, op1=ALU.add
                        )
                        # keep the off-chain u/m2 behind q in the DVE stream so
                        # they fill the tanh window instead of delaying it
                        _add_dep_helper(i_u.ins, i_q[ch].ins, sync=False,
                                        reason="order u after q")
                        m2 = sp.tile([H, BC], F32, tag=f"m2{ch}", name=f"m2{ch}")
                        i_m2 = nc.vector.tensor_mul(m2, ps_z[ch], h_aug[ch][0:H])
                        _add_dep_helper(i_m2.ins, i_u.ins, sync=False,
                                        reason="order m2 after u")
                        m1 = sp.tile([H, BC], F32, tag=f"m1{ch}", name=f"m1{ch}")
                        nc.vector.tensor_mul(m1, u, nt[ch])
                        nc.vector.tensor_add(h_aug[ch][0:H], m1, m2)
                    for ch in range(NCH):
                        # snapshot h_t (feeds history DMA, logits, next d)
                        hs = snapp.tile([H, BC], F32, tag=f"hs{ch}")
                        nc.gpsimd.tensor_copy(hs, h_aug[ch][0:H])
                        nc.sync.dma_start(
                            out=hist[t % 128 : t % 128 + 1, t // 128, ch, :],
                            in_=hs,
                        )
                        if t % 32 == 0:
                            ps_l[ch] = psp2.tile([1, BC * 32], F32, tag=f"psl{ch}", name=f"ps_l{ch}")
                        nc.tensor.matmul(
                            ps_l[ch][:, (t % 32) * BC : (t % 32 + 1) * BC],
                            lhsT=wattn, rhs=h_aug[ch][0:H], start=True, stop=True,
                        )
                        if t % 32 == 31:
                            blk = t // 32
                            l_sb = sp.tile([1, BC * 32], F32, tag=f"lsb{ch}")
                            nc.scalar.activation(l_sb, ps_l[ch], AF.Identity)
                            nc.sync.dma_start(
                                out=l_ds[ch].ap()[
                                    :, blk * BC * 32 : (blk + 1) * BC * 32
                                ],
                                in_=l_sb,
                            )
                    if t % 16 == 15:
                        for _ in range(2):
                            if next_chunk < n_ck:
                                emit_gemm_chunk(next_chunk)
                                next_chunk += 1

            psp12_cm.__exit__(None, None, None)

            # ---- phase 3: softmax + context + fc ----
            with tc.tile_pool(name="ps3", bufs=2, space="PSUM") as psp3:
                l_bt = p3.tile([B, S], F32)
                for ch in range(NCH):
                    nc.sync.dma_start(
                        out=l_bt[ch * BC : (ch + 1) * BC],
                        in_=l_ds[ch].ap().rearrange("o (s b) -> (o b) s", b=BC),
                    )
                mx = p3.tile([B, 1], F32)
                nc.vector.reduce_max(mx, l_bt, axis=mybir.AxisListType.X, negate=True)
                e_bt = p3.tile([B, S], F32)
                ssum = p3.tile([B, 1], F32)
                nc.scalar.activation(
                    e_bt, l_bt, AF.Exp, bias=mx, scale=1.0, accum_out=ssum
                )
                rinv = p3.tile([B, 1], F32)
                nc.vector.reciprocal(rinv, ssum)
                attn = p3.tile([B, S], F32)
                nc.vector.tensor_scalar_mul(attn, e_bt, rinv)

                attn_tb = []
                for c in range(n_tchunk):
                    ps_tr = psp3.tile([128, B], F32, tag="pstr")
                    nc.tensor.transpose(
                        ps_tr, attn[:, c * 128 : (c + 1) * 128], ident[0:B, 0:B]
                    )
                    a_tb = p3.tile([128, B], F32, tag=f"atb{c}")
                    nc.vector.tensor_copy(a_tb, ps_tr)
                    attn_tb.append(a_tb)

                hist_v = hist.rearrange("p c ch (h b) -> p c ch h b", b=BC)
                ctx_ps = psp3.tile([H, B], F32, tag="ctx")
                for b in range(B):
                    ch, b16 = b // BC, b % BC
                    for c in range(n_tchunk):
                        nc.tensor.matmul(
                            ctx_ps[:, b : b + 1],
                            lhsT=hist_v[:, c, ch, :, b16],
                            rhs=attn_tb[c][:, b : b + 1],
                            start=(c == 0),
                            stop=(c == n_tchunk - 1),
                        )
                ctx_aug = p3.tile([H + 1, B], F32)
                nc.vector.memset(ctx_aug[H : H + 1], 1.0)
                nc.vector.tensor_copy(ctx_aug[0:H], ctx_ps)
                y_ps = psp3.tile([C, B], F32, tag="y")
                nc.tensor.matmul(y_ps, lhsT=wfc, rhs=ctx_aug, start=True, stop=True)
                y_sb = p3.tile([C, B], F32)
                nc.vector.tensor_copy(y_sb, y_ps)
                nc.sync.dma_start(out=y_d.ap().rearrange("b c -> c b"), in_=y_sb)

    nc.compile()
    return nc


def prep_core_inputs(x_shard, w_ih, w_hh, b_ih, b_hh, w_attn, w_fc, b_fc):
    """Build the per-core in_map from a [B, S, I] f32 shard + full params."""
    B, S, I_ = x_shard.shape
    # t-major token order [i, t*B + b]: phase-2's step-t slice is contiguous
    # and phase 1 produces early timesteps first (lets the recurrence start
    # while the input GEMM tail is still running)
    xT = np.ascontiguousarray(
        x_shard.transpose(2, 1, 0).reshape(I_, B * S), dtype=np.float32
    )
    w_hhT_aug = np.zeros((H + 1, G), dtype=np.float32)
    w_hhT_aug[0:H, :] = w_hh.T
    w_hhT_aug[H, 2 * H : G] = b_hh[2 * H : G]  # b_hh_n via ones-row
    bias_rz = (b_ih[0 : 2 * H] + b_hh[0 : 2 * H]).reshape(2 * H, 1)
    bias_n = b_ih[2 * H : G].reshape(H, 1)
    w_fcT_aug = np.zeros((H + 1, C), dtype=np.float32)
    w_fcT_aug[0:H, :] = w_fc.T
    w_fcT_aug[H, :] = b_fc
    return {
        "xT": xT,
        "w_ihT": np.ascontiguousarray(w_ih.T, dtype=np.float32),
        "w_hhT_aug": w_hhT_aug,
        "bias_rz": np.ascontiguousarray(bias_rz, dtype=np.float32),
        "bias_n": np.ascontiguousarray(bias_n, dtype=np.float32),
        "ident": np.eye(128, dtype=np.float32),
        "w_attn_col": np.ascontiguousarray(w_attn.T, dtype=np.float32),
        "w_fcT_aug": w_fcT_aug,
    }


_NC_CACHE = {}


def kernel(x, w_ih, w_hh, b_ih, b_hh, w_attn, b_attn, w_fc, b_fc):
    x = np.asarray(x, dtype=np.float32)
    w_ih = np.asarray(w_ih, dtype=np.float32)
    w_hh = np.asarray(w_hh, dtype=np.float32)
    b_ih = np.asarray(b_ih, dtype=np.float32)
    b_hh = np.asarray(b_hh, dtype=np.float32)
    w_attn = np.asarray(w_attn, dtype=np.float32)
    w_fc = np.asarray(w_fc, dtype=np.float32)
    b_fc = np.asarray(b_fc, dtype=np.float32)

    Bfull, S, _ = x.shape
    B = Bfull // N_CORES
    key = (S, B)
    if key not in _NC_CACHE:
        _NC_CACHE[key] = build_program(S, B, num_devices=N_CORES)
    nc = _NC_CACHE[key]

    in_maps = []
    for c in range(N_CORES):
        shard = x[c * B : (c + 1) * B]
        in_maps.append(
            prep_core_inputs(shard, w_ih, w_hh, b_ih, b_hh, w_attn, w_fc, b_fc)
        )
    res = bass_utils.run_bass_kernel_spmd(nc, in_maps, core_ids=list(range(N_CORES)))
    out = np.concatenate([res.results[c]["y"] for c in range(N_CORES)], axis=0)
    return out.astype(np.float32)
